# revision 28
# baseline (speedup 1.0000x reference)
"""Trainium2 Bass kernel for single-head attention with QKV projections.

Problem: q,k,v [4, 2048, 1024] fp32; w_q/w_k/w_v [1024, 1024]; b_* [1024];
additive mask [1, 2048, 2048].
  query = q @ w_q.T + b_q ; key = k @ w_k.T + b_k ; value = v @ w_v.T + b_v
  att = softmax(query @ key.T / sqrt(D) + mask) ; out = att @ value

Sharding: 8 cores = 4 batches x 2 sequence-halves of q rows (1024 rows per
core). Each core computes the full K/V projection for its batch (duplicated
across the pair) and the attention output for its q rows. Uniform SPMD
program; the mask is applied as data (no causality assumption).

Host-side prep (free, not on-device): transposes/blocks q/k/v and the
weights so the contraction dim lands on SBUF partitions and every DMA is
contiguous per partition, and pre-scales the mask by sqrt(D) so it can be
added to the raw QK^T product before the 1/sqrt(D) scaling fused into the
exp activation.

Matmul operands are float32r (fp32 bits, reduced-precision multiplier,
fp32 PSUM accumulation): ~4x the fp32 matmul throughput at ~2e-4 relative
output error (vs 2e-6 all-fp32).
"""

import math

import numpy as np

import concourse.bass as bass
import concourse.mybir as mybir
import concourse.tile as tile
from concourse import bacc
from concourse.bass_utils import run_bass_kernel_spmd
from concourse.masks import make_identity

B, S, D = 4, 2048, 1024
SQ = S // 2          # q rows per core
P = 128              # partitions
NE = D // P          # 8 feature blocks
NQT = SQ // P        # 8 q tiles per core
NKB = S // P         # 16 key blocks
KC = 512             # key chunk for QK^T matmuls
NKC = S // KC        # 4 key chunks
SCALE = 1.0 / math.sqrt(D)

F32 = mybir.dt.float32
MM_DT = mybir.dt.float32r


def build_bass():
    # Bacc (not raw Bass): its compile() pass legalizes semaphore waits
    # (move_matmul_waits_to_ldweights + generate_event_semaphores) for the
    # TRN2 1-wait-per-instruction constraint.
    nc = bacc.Bacc("TRN2", target_bir_lowering=False, debug=False, num_devices=8)

    # Activations pre-blocked so each SBUF tile load is one contiguous run
    # per partition: index [chunk][p=d_in][o=d_blk][s].
    qT = nc.dram_tensor("qT", [SQ // KC, P, NE, KC], MM_DT, kind="ExternalInput")
    kT = nc.dram_tensor("kT", [NKC, P, NE, KC], MM_DT, kind="ExternalInput")
    vT = nc.dram_tensor("vT", [NKB, P, NE, P], MM_DT, kind="ExternalInput")
    # Weight e-panels, same blocking: [panel][p=d_in][o=d_blk][e].
    wqT = nc.dram_tensor("wqT", [NE, P, NE, P], MM_DT, kind="ExternalInput")
    wkT = nc.dram_tensor("wkT", [NE, P, NE, P], MM_DT, kind="ExternalInput")
    wvT = nc.dram_tensor("wvT", [2, P, NE, KC], MM_DT, kind="ExternalInput")
    bq = nc.dram_tensor("bq", [D], F32, kind="ExternalInput")
    bk = nc.dram_tensor("bk", [D], F32, kind="ExternalInput")
    bv = nc.dram_tensor("bv", [D], F32, kind="ExternalInput")
    maskS = nc.dram_tensor("maskS", [SQ, S], F32, kind="ExternalInput")
    out = nc.dram_tensor("out", [SQ, D], F32, kind="ExternalOutput")

    bq2 = bq.rearrange("(o p) -> p o", p=P)
    bk2 = bk.rearrange("(o p) -> p o", p=P)

    with tile.TileContext(nc) as tc:
        with (
            tc.tile_pool(name="const", bufs=1) as const_pool,
            tc.tile_pool(name="qk_res", bufs=1) as qk_res,
            tc.tile_pool(name="ps_att", bufs=4, space="PSUM") as ps_att,
        ):
            identity = const_pool.tile([P, P], F32)
            make_identity(nc, identity)
            bq_sb = const_pool.tile([P, NE], F32, tag="bq")
            nc.gpsimd.dma_start(out=bq_sb, in_=bq2)
            bk_sb = const_pool.tile([P, NE], F32, tag="bk")
            nc.gpsimd.dma_start(out=bk_sb, in_=bk2)

            # Resident: queryT (32KB/part) + keyT (64KB/part)
            queryT_sb = qk_res.tile([P, NE, SQ], MM_DT, tag="queryT")
            keyT_sb = qk_res.tile([P, NE, S], MM_DT, tag="keyT")

            # ---- Phases 1+2: Q then K projections (transposed outputs).
            # Weight e-panels stream on the scalar HWDGE queue (parallel to
            # activations on sync). All s-chunks of the input stay resident;
            # for eb==0 the matmuls run s-chunk-major so early matmuls only
            # need the first chunk; later eb run chunk-minor so one
            # stationary w-panel block feeds n_sc back-to-back matmuls.
            ps_proj_cm = tc.tile_pool(name="ps_proj", bufs=4, space="PSUM")
            ps_proj = ps_proj_cm.__enter__()
            with (
                tc.tile_pool(name="wpan", bufs=2) as wpan_pool,
                tc.tile_pool(name="ins", bufs=4) as in_pool,
            ):
                # Emit Q then K. The sync queue is FIFO, so the first two
                # K input chunks are queued during the Q phase (their ins
                # slots are free), keeping the K-phase start fed.
                def proj_phase(w4, x4, b_sb, dst, n_sc, xs, w0):
                    for eb in range(NE):
                        if eb == 0:
                            w_t = w0
                        else:
                            w_t = wpan_pool.tile(
                                [P, NE, P], MM_DT, tag="wpan", name="w_t"
                            )
                            nc.sync.dma_start(out=w_t, in_=w4[eb])
                        pss = [
                            ps_proj.tile([P, KC], F32, name="ps", tag="ps")
                            for _ in range(n_sc)
                        ]
                        if eb == 0:
                            order = [(sc, db) for sc in range(n_sc) for db in range(NE)]
                        else:
                            order = [(sc, db) for db in range(NE) for sc in range(n_sc)]
                        for sc, db in order:
                            nc.tensor.matmul(
                                pss[sc],
                                w_t[:, db, :],
                                xs[sc][:, db, :],
                                start=(db == 0),
                                stop=(db == NE - 1),
                            )
                        for sc in range(n_sc):
                            nc.scalar.activation(
                                out=dst[:, eb, sc * KC:(sc + 1) * KC],
                                in_=pss[sc],
                                func=mybir.ActivationFunctionType.Identity,
                                bias=b_sb[:, eb:eb + 1],
                            )

                def load_chunk(x4, sc):
                    x_t = in_pool.tile([P, NE, KC], MM_DT, tag="ins", name="x_t")
                    nc.sync.dma_start(out=x_t, in_=x4[sc])
                    return x_t

                wq0 = wpan_pool.tile([P, NE, P], MM_DT, tag="wpan", name="wq0")
                nc.sync.dma_start(out=wq0, in_=wqT[0])
                qs = [load_chunk(qT, sc) for sc in range(SQ // KC)]
                ks01 = [load_chunk(kT, sc) for sc in range(2)]
                proj_phase(wqT, qT, bq_sb, queryT_sb, SQ // KC, qs, wq0)

                ks23 = [load_chunk(kT, sc) for sc in range(2, NKC)]
                wk0 = wpan_pool.tile([P, NE, P], MM_DT, tag="wpan", name="wk0")
                nc.sync.dma_start(out=wk0, in_=wkT[0])
                proj_phase(wkT, kT, bk_sb, keyT_sb, NKC, ks01 + ks23, wk0)

            with tc.tile_pool(name="v_res", bufs=1) as v_res:
                value_sb = v_res.tile([P, NKB, D], MM_DT, tag="value")

                # ---- Phase 3: V projection -> value_sb [k, e] (natural)
                with (
                    tc.tile_pool(name="wv", bufs=2) as wv_pool,
                    tc.tile_pool(name="vins", bufs=3) as vin_pool,
                ):
                    for ec in range(2):
                        wv_sb = wv_pool.tile([P, NE, KC], MM_DT, tag="wv", name="wv_sb")
                        nc.sync.dma_start(out=wv_sb, in_=wvT[ec])
                        for kb in range(NKB):
                            vs = vin_pool.tile([P, NE, P], MM_DT, tag="vins", name="vs")
                            nc.sync.dma_start(out=vs, in_=vT[kb])
                            ps = ps_proj.tile([P, KC], F32, name="ps", tag="ps")
                            for db in range(NE):
                                nc.tensor.matmul(
                                    ps,
                                    vs[:, db, :],
                                    wv_sb[:, db, :],
                                    start=(db == 0),
                                    stop=(db == NE - 1),
                                )
                            # bias b_v is added at the output eviction:
                            # softmax rows sum to 1, so out += b_v exactly.
                            nc.scalar.copy(
                                out=value_sb[:, kb, ec * KC:(ec + 1) * KC],
                                in_=ps,
                            )

                ps_proj_cm.__exit__(None, None, None)

                # ---- Phase 4: attention, software-pipelined per q tile:
                # PE order is QK(0), QK(1), TR/PV(0), QK(2), TR/PV(1), ...
                # so the softmax chain (DVE/ACT) of tile j overlaps QK(j+1).
                with (
                    tc.tile_pool(name="z", bufs=1) as z_pool,
                    tc.tile_pool(name="p", bufs=1) as p_pool,
                    tc.tile_pool(name="mask", bufs=8) as mask_pool,
                    tc.tile_pool(name="pt", bufs=3) as pt_pool,
                    tc.tile_pool(name="stats", bufs=4) as stat_pool,
                    tc.tile_pool(name="outs", bufs=1) as out_pool,
                    tc.tile_pool(name="ps_tr", bufs=2, space="PSUM") as ps_tr,
                    tc.tile_pool(name="ps_out", bufs=1, space="PSUM") as ps_out,
                ):
                    bv_bcast = out_pool.tile([P, D], F32, tag="bv")
                    nc.gpsimd.dma_start(
                        out=bv_bcast, in_=bv[None, :].to_broadcast([P, D])
                    )

                    def emit_qk_softmax(j):
                        pss_a = [
                            ps_att.tile([P, KC], F32, name="ps_a", tag="ps_a")
                            for _ in range(NKC)
                        ]
                        for eb in range(NE):
                            for kc in range(NKC):
                                nc.tensor.matmul(
                                    pss_a[kc],
                                    queryT_sb[:, eb, j * P:(j + 1) * P],
                                    keyT_sb[:, eb, kc * KC:(kc + 1) * KC],
                                    start=(eb == 0),
                                    stop=(eb == NE - 1),
                                )
                        z_sb = z_pool.tile([P, S], F32, tag="z", name="z_sb")
                        for kc in range(NKC):
                            mask_t = mask_pool.tile(
                                [P, KC], F32, tag="mask", name="mask_t"
                            )
                            nc.gpsimd.dma_start(
                                out=mask_t,
                                in_=maskS[j * P:(j + 1) * P, kc * KC:(kc + 1) * KC],
                            )
                            # z = raw QK^T + mask*sqrt(D)
                            nc.vector.tensor_add(
                                out=z_sb[:, kc * KC:(kc + 1) * KC],
                                in0=pss_a[kc],
                                in1=mask_t,
                            )
                        m_t = stat_pool.tile([P, 1], F32, tag="m", name="m_t")
                        nc.vector.reduce_max(m_t, z_sb, axis=mybir.AxisListType.X)
                        negm = stat_pool.tile([P, 1], F32, tag="negm", name="negm")
                        nc.vector.tensor_scalar_mul(out=negm, in0=m_t, scalar1=-SCALE)
                        l_t = stat_pool.tile([P, 1], F32, tag="l", name="l_t")
                        p_sb = p_pool.tile([P, S], F32, tag="p", name="p_sb")
                        # p = exp(z/sqrt(D) - m/sqrt(D)); l = rowsum(p)
                        nc.scalar.activation(
                            out=p_sb,
                            in_=z_sb,
                            func=mybir.ActivationFunctionType.Exp,
                            bias=negm,
                            scale=SCALE,
                            accum_out=l_t,
                        )
                        recip_l = stat_pool.tile([P, 1], F32, tag="recip", name="recip")
                        nc.vector.reciprocal(recip_l, l_t)
                        return j, p_sb, recip_l

                    def emit_pv(j, p_sb, recip_l):
                        ps_o = ps_out.tile([P, D], F32, name="ps_o", tag="out")
                        for kb in range(NKB):
                            ps_t = ps_tr.tile([P, P], F32, name="ps_t", tag="tr")
                            nc.tensor.transpose(
                                ps_t, p_sb[:, kb * P:(kb + 1) * P], identity
                            )
                            pT_sb = pt_pool.tile([P, P], MM_DT, tag="pt", name="pT_sb")
                            nc.scalar.copy(out=pT_sb, in_=ps_t)
                            for ec in range(2):
                                nc.tensor.matmul(
                                    ps_o[:, ec * KC:(ec + 1) * KC],
                                    pT_sb,
                                    value_sb[:, kb, ec * KC:(ec + 1) * KC],
                                    start=(kb == 0),
                                    stop=(kb == NKB - 1),
                                )
                        out_sb = out_pool.tile([P, D], F32, tag="out", name="out_sb")
                        # out = (p@v_raw)@w_v.T / l + b_v
                        nc.vector.scalar_tensor_tensor(
                            out=out_sb,
                            in0=ps_o,
                            scalar=recip_l,
                            in1=bv_bcast,
                            op0=mybir.AluOpType.mult,
                            op1=mybir.AluOpType.add,
                        )
                        nc.sync.dma_start(out=out[j * P:(j + 1) * P, :], in_=out_sb)

                    state = emit_qk_softmax(0)
                    for j in range(1, NQT):
                        nxt = emit_qk_softmax(j)
                        emit_pv(*state)
                        state = nxt
                    emit_pv(*state)

    nc.finalize()
    return nc


_NC_CACHE = None
LAST_RESULT = None  # BassKernelResults from the most recent kernel() call


def _block_xT(x, chunk):
    """[s_total, D] activation -> [s_total/chunk, P, NE, chunk] d-major blocks.

    Result[c, p, o, s] = x[c*chunk + s, o*P + p] — x.T chunked along s with
    the 1024-wide d axis split into NE partition blocks; each chunk is
    contiguous per partition for single-run DMA descriptors.
    """
    nchunk = x.shape[0] // chunk
    return np.ascontiguousarray(
        x.reshape(nchunk, chunk, NE, P).transpose(0, 3, 2, 1)
    )


def _block_w_panels(wT, panel):
    """[D, D] pre-transposed weight -> [D/panel, P, NE, panel] e-panels."""
    n = wT.shape[1] // panel
    return np.ascontiguousarray(
        wT.reshape(NE, P, n, panel).transpose(2, 1, 0, 3)
    )


def kernel(q, k, v, mask, w_q, b_q, w_k, b_k, w_v, b_v):
    global _NC_CACHE, LAST_RESULT
    if _NC_CACHE is None:
        _NC_CACHE = build_bass()
    nc = _NC_CACHE

    f32 = np.float32
    wqT = _block_w_panels(np.asarray(w_q, dtype=f32).T, P)
    wkT = _block_w_panels(np.asarray(w_k, dtype=f32).T, P)
    wvT = _block_w_panels(np.asarray(w_v, dtype=f32).T, KC)
    bq = np.ascontiguousarray(np.asarray(b_q, dtype=f32))
    bk = np.ascontiguousarray(np.asarray(b_k, dtype=f32))
    bv = np.ascontiguousarray(np.asarray(b_v, dtype=f32))
    mask = np.asarray(mask, dtype=f32)
    # pre-scale so the kernel can add it to raw QK^T before the fused 1/sqrt(D)
    maskS_halves = [
        np.ascontiguousarray(mask[0, h * SQ:(h + 1) * SQ, :] * f32(math.sqrt(D)))
        for h in range(2)
    ]

    kT_b = [_block_xT(np.asarray(k[b], dtype=f32), KC) for b in range(B)]
    vT_b = [_block_xT(np.asarray(v[b], dtype=f32), P) for b in range(B)]

    in_maps = []
    for c in range(8):
        b, h = c // 2, c % 2
        rows = slice(h * SQ, (h + 1) * SQ)
        in_maps.append({
            "qT": _block_xT(np.asarray(q[b], dtype=f32)[rows, :], KC),
            "kT": kT_b[b],
            "vT": vT_b[b],
            "wqT": wqT, "wkT": wkT, "wvT": wvT,
            "bq": bq, "bk": bk, "bv": bv,
            "maskS": maskS_halves[h],
        })

    res = run_bass_kernel_spmd(nc, in_maps, list(range(8)))
    LAST_RESULT = res

    out = np.empty((B, S, D), dtype=f32)
    for c in range(8):
        b, h = c // 2, c % 2
        out[b, h * SQ:(h + 1) * SQ, :] = res.results[c]["out"]
    return out


# revision 29
# speedup vs baseline: 1.0664x; 1.0664x over previous
"""Trainium2 Bass kernel for single-head attention with QKV projections.

Problem: q,k,v [4, 2048, 1024] fp32; w_q/w_k/w_v [1024, 1024]; b_* [1024];
additive mask [1, 2048, 2048].
  query = q @ w_q.T + b_q ; key = k @ w_k.T + b_k ; value = v @ w_v.T + b_v
  att = softmax(query @ key.T / sqrt(D) + mask) ; out = att @ value

Sharding: 8 cores = 4 batches x 2 sequence-halves of q rows (1024 rows per
core). Each core computes the full K/V projection for its batch (duplicated
across the pair) and the attention output for its q rows. Uniform SPMD
program; the mask is applied as data (no causality assumption).

Host-side prep (free, not on-device): transposes/blocks q/k/v and the
weights so the contraction dim lands on SBUF partitions and every DMA is
contiguous per partition, and pre-scales the mask by sqrt(D) so it can be
added to the raw QK^T product before the 1/sqrt(D) scaling fused into the
exp activation.

Matmul operands are float32r (fp32 bits, reduced-precision multiplier,
fp32 PSUM accumulation): ~4x the fp32 matmul throughput at ~2e-4 relative
output error (vs 2e-6 all-fp32).
"""

import math

import numpy as np

import concourse.bass as bass
import concourse.mybir as mybir
import concourse.tile as tile
from concourse import bacc
from concourse.bass_utils import run_bass_kernel_spmd
from concourse.masks import make_identity

B, S, D = 4, 2048, 1024
SQ = S // 2          # q rows per core
P = 128              # partitions
NE = D // P          # 8 feature blocks
NQT = SQ // P        # 8 q tiles per core
NKB = S // P         # 16 key blocks
KC = 512             # key chunk for QK^T matmuls
NKC = S // KC        # 4 key chunks
SCALE = 1.0 / math.sqrt(D)

F32 = mybir.dt.float32
MM_DT = mybir.dt.float32r


def build_bass():
    # Bacc (not raw Bass): its compile() pass legalizes semaphore waits
    # (move_matmul_waits_to_ldweights + generate_event_semaphores) for the
    # TRN2 1-wait-per-instruction constraint.
    nc = bacc.Bacc("TRN2", target_bir_lowering=False, debug=False, num_devices=8)

    # Activations pre-blocked so each SBUF tile load is one contiguous run
    # per partition: index [chunk][p=d_in][o=d_blk][s].
    qT = nc.dram_tensor("qT", [SQ // KC, P, NE, KC], MM_DT, kind="ExternalInput")
    kT = nc.dram_tensor("kT", [NKC, P, NE, KC], MM_DT, kind="ExternalInput")
    vT = nc.dram_tensor("vT", [NKB, P, NE, P], MM_DT, kind="ExternalInput")
    # Weight e-panels, same blocking: [panel][p=d_in][o=d_blk][e].
    wqT = nc.dram_tensor("wqT", [NE, P, NE, P], MM_DT, kind="ExternalInput")
    wkT = nc.dram_tensor("wkT", [NE, P, NE, P], MM_DT, kind="ExternalInput")
    wvT = nc.dram_tensor("wvT", [2, P, NE, KC], MM_DT, kind="ExternalInput")
    bq = nc.dram_tensor("bq", [D], F32, kind="ExternalInput")
    bk = nc.dram_tensor("bk", [D], F32, kind="ExternalInput")
    bv = nc.dram_tensor("bv", [D], F32, kind="ExternalInput")
    maskS = nc.dram_tensor("maskS", [SQ, S], F32, kind="ExternalInput")
    out = nc.dram_tensor("out", [SQ, D], F32, kind="ExternalOutput")

    bq2 = bq.rearrange("(o p) -> p o", p=P)
    bk2 = bk.rearrange("(o p) -> p o", p=P)

    with tile.TileContext(nc) as tc:
        with (
            tc.tile_pool(name="const", bufs=1) as const_pool,
            tc.tile_pool(name="qk_res", bufs=1) as qk_res,
            tc.tile_pool(name="ps_att", bufs=4, space="PSUM") as ps_att,
        ):
            identity = const_pool.tile([P, P], F32)
            make_identity(nc, identity)
            bq_sb = const_pool.tile([P, NE], F32, tag="bq")
            nc.gpsimd.dma_start(out=bq_sb, in_=bq2)
            bk_sb = const_pool.tile([P, NE], F32, tag="bk")
            nc.gpsimd.dma_start(out=bk_sb, in_=bk2)

            # Resident: queryT (32KB/part) + keyT (64KB/part)
            queryT_sb = qk_res.tile([P, NE, SQ], MM_DT, tag="queryT")
            keyT_sb = qk_res.tile([P, NE, S], MM_DT, tag="keyT")

            # ---- Phases 1+2: Q then K projections (transposed outputs).
            # Weight e-panels stream on the scalar HWDGE queue (parallel to
            # activations on sync). All s-chunks of the input stay resident;
            # for eb==0 the matmuls run s-chunk-major so early matmuls only
            # need the first chunk; later eb run chunk-minor so one
            # stationary w-panel block feeds n_sc back-to-back matmuls.
            ps_proj_cm = tc.tile_pool(name="ps_proj", bufs=4, space="PSUM")
            ps_proj = ps_proj_cm.__enter__()
            with (
                tc.tile_pool(name="wpan", bufs=2) as wpan_pool,
                tc.tile_pool(name="ins", bufs=4) as in_pool,
            ):
                for which in ("q", "k"):
                    w4, x4, b_sb, dst, n_sc = {
                        "q": (wqT, qT, bq_sb, queryT_sb, SQ // KC),
                        "k": (wkT, kT, bk_sb, keyT_sb, NKC),
                    }[which]
                    w0 = wpan_pool.tile([P, NE, P], MM_DT, tag="wpan", name="w0")
                    nc.sync.dma_start(out=w0, in_=w4[0])
                    xs = []
                    for sc in range(n_sc):
                        x_t = in_pool.tile([P, NE, KC], MM_DT, tag="ins", name="x_t")
                        nc.sync.dma_start(out=x_t, in_=x4[sc])
                        xs.append(x_t)
                    for eb in range(NE):
                        if eb == 0:
                            w_t = w0
                        else:
                            w_t = wpan_pool.tile(
                                [P, NE, P], MM_DT, tag="wpan", name="w_t"
                            )
                            nc.sync.dma_start(out=w_t, in_=w4[eb])
                        pss = [
                            ps_proj.tile([P, KC], F32, name="ps", tag="ps")
                            for _ in range(n_sc)
                        ]
                        if eb == 0:
                            order = [(sc, db) for sc in range(n_sc) for db in range(NE)]
                        else:
                            order = [(sc, db) for db in range(NE) for sc in range(n_sc)]
                        for sc, db in order:
                            nc.tensor.matmul(
                                pss[sc],
                                w_t[:, db, :],
                                xs[sc][:, db, :],
                                start=(db == 0),
                                stop=(db == NE - 1),
                            )
                        for sc in range(n_sc):
                            nc.scalar.activation(
                                out=dst[:, eb, sc * KC:(sc + 1) * KC],
                                in_=pss[sc],
                                func=mybir.ActivationFunctionType.Identity,
                                bias=b_sb[:, eb:eb + 1],
                            )

            with tc.tile_pool(name="v_res", bufs=1) as v_res:
                value_sb = v_res.tile([P, NKB, D], MM_DT, tag="value")

                # ---- Phase 3: V projection -> value_sb [k, e] (natural)
                with (
                    tc.tile_pool(name="wv", bufs=2) as wv_pool,
                    tc.tile_pool(name="vins", bufs=3) as vin_pool,
                ):
                    for ec in range(2):
                        wv_sb = wv_pool.tile([P, NE, KC], MM_DT, tag="wv", name="wv_sb")
                        nc.sync.dma_start(out=wv_sb, in_=wvT[ec])
                        for kb in range(NKB):
                            vs = vin_pool.tile([P, NE, P], MM_DT, tag="vins", name="vs")
                            nc.sync.dma_start(out=vs, in_=vT[kb])
                            ps = ps_proj.tile([P, KC], F32, name="ps", tag="ps")
                            for db in range(NE):
                                nc.tensor.matmul(
                                    ps,
                                    vs[:, db, :],
                                    wv_sb[:, db, :],
                                    start=(db == 0),
                                    stop=(db == NE - 1),
                                )
                            # bias b_v is added at the output eviction:
                            # softmax rows sum to 1, so out += b_v exactly.
                            nc.scalar.copy(
                                out=value_sb[:, kb, ec * KC:(ec + 1) * KC],
                                in_=ps,
                            )

                ps_proj_cm.__exit__(None, None, None)

                # ---- Phase 4: attention, software-pipelined per q tile:
                # PE order is QK(0), QK(1), TR/PV(0), QK(2), TR/PV(1), ...
                # so the softmax chain (DVE/ACT) of tile j overlaps QK(j+1).
                with (
                    tc.tile_pool(name="z", bufs=1) as z_pool,
                    tc.tile_pool(name="p", bufs=1) as p_pool,
                    tc.tile_pool(name="mask", bufs=8) as mask_pool,
                    tc.tile_pool(name="pt", bufs=3) as pt_pool,
                    tc.tile_pool(name="stats", bufs=4) as stat_pool,
                    tc.tile_pool(name="outs", bufs=1) as out_pool,
                    tc.tile_pool(name="ps_tr", bufs=2, space="PSUM") as ps_tr,
                    tc.tile_pool(name="ps_out", bufs=1, space="PSUM") as ps_out,
                ):
                    bv_bcast = out_pool.tile([P, D], F32, tag="bv")
                    nc.gpsimd.dma_start(
                        out=bv_bcast, in_=bv[None, :].to_broadcast([P, D])
                    )

                    def emit_qk_softmax(j):
                        pss_a = [
                            ps_att.tile([P, KC], F32, name="ps_a", tag="ps_a")
                            for _ in range(NKC)
                        ]
                        for eb in range(NE):
                            for kc in range(NKC):
                                nc.tensor.matmul(
                                    pss_a[kc],
                                    queryT_sb[:, eb, j * P:(j + 1) * P],
                                    keyT_sb[:, eb, kc * KC:(kc + 1) * KC],
                                    start=(eb == 0),
                                    stop=(eb == NE - 1),
                                )
                        z_sb = z_pool.tile([P, S], F32, tag="z", name="z_sb")
                        for kc in range(NKC):
                            mask_t = mask_pool.tile(
                                [P, KC], F32, tag="mask", name="mask_t"
                            )
                            nc.gpsimd.dma_start(
                                out=mask_t,
                                in_=maskS[j * P:(j + 1) * P, kc * KC:(kc + 1) * KC],
                            )
                            # z = raw QK^T + mask*sqrt(D)
                            nc.vector.tensor_add(
                                out=z_sb[:, kc * KC:(kc + 1) * KC],
                                in0=pss_a[kc],
                                in1=mask_t,
                            )
                        m_t = stat_pool.tile([P, 1], F32, tag="m", name="m_t")
                        nc.vector.reduce_max(m_t, z_sb, axis=mybir.AxisListType.X)
                        negm = stat_pool.tile([P, 1], F32, tag="negm", name="negm")
                        nc.vector.tensor_scalar_mul(out=negm, in0=m_t, scalar1=-SCALE)
                        l_t = stat_pool.tile([P, 1], F32, tag="l", name="l_t")
                        p_sb = p_pool.tile([P, S], F32, tag="p", name="p_sb")
                        # p = exp(z/sqrt(D) - m/sqrt(D)); l = rowsum(p)
                        nc.scalar.activation(
                            out=p_sb,
                            in_=z_sb,
                            func=mybir.ActivationFunctionType.Exp,
                            bias=negm,
                            scale=SCALE,
                            accum_out=l_t,
                        )
                        recip_l = stat_pool.tile([P, 1], F32, tag="recip", name="recip")
                        nc.vector.reciprocal(recip_l, l_t)
                        return j, p_sb, recip_l

                    def emit_pv(j, p_sb, recip_l):
                        ps_o = ps_out.tile([P, D], F32, name="ps_o", tag="out")
                        for kb in range(NKB):
                            ps_t = ps_tr.tile([P, P], F32, name="ps_t", tag="tr")
                            nc.tensor.transpose(
                                ps_t, p_sb[:, kb * P:(kb + 1) * P], identity
                            )
                            pT_sb = pt_pool.tile([P, P], MM_DT, tag="pt", name="pT_sb")
                            nc.scalar.copy(out=pT_sb, in_=ps_t)
                            for ec in range(2):
                                nc.tensor.matmul(
                                    ps_o[:, ec * KC:(ec + 1) * KC],
                                    pT_sb,
                                    value_sb[:, kb, ec * KC:(ec + 1) * KC],
                                    start=(kb == 0),
                                    stop=(kb == NKB - 1),
                                )
                        out_sb = out_pool.tile([P, D], F32, tag="out", name="out_sb")
                        # out = (p@v_raw)@w_v.T / l + b_v
                        nc.vector.scalar_tensor_tensor(
                            out=out_sb,
                            in0=ps_o,
                            scalar=recip_l,
                            in1=bv_bcast,
                            op0=mybir.AluOpType.mult,
                            op1=mybir.AluOpType.add,
                        )
                        nc.sync.dma_start(out=out[j * P:(j + 1) * P, :], in_=out_sb)

                    state = emit_qk_softmax(0)
                    for j in range(1, NQT):
                        nxt = emit_qk_softmax(j)
                        emit_pv(*state)
                        state = nxt
                    emit_pv(*state)

    nc.finalize()
    return nc


_NC_CACHE = None
LAST_RESULT = None  # BassKernelResults from the most recent kernel() call


def _block_xT(x, chunk):
    """[s_total, D] activation -> [s_total/chunk, P, NE, chunk] d-major blocks.

    Result[c, p, o, s] = x[c*chunk + s, o*P + p] — x.T chunked along s with
    the 1024-wide d axis split into NE partition blocks; each chunk is
    contiguous per partition for single-run DMA descriptors.
    """
    nchunk = x.shape[0] // chunk
    return np.ascontiguousarray(
        x.reshape(nchunk, chunk, NE, P).transpose(0, 3, 2, 1)
    )


def _block_w_panels(wT, panel):
    """[D, D] pre-transposed weight -> [D/panel, P, NE, panel] e-panels."""
    n = wT.shape[1] // panel
    return np.ascontiguousarray(
        wT.reshape(NE, P, n, panel).transpose(2, 1, 0, 3)
    )


def kernel(q, k, v, mask, w_q, b_q, w_k, b_k, w_v, b_v):
    global _NC_CACHE, LAST_RESULT
    if _NC_CACHE is None:
        _NC_CACHE = build_bass()
    nc = _NC_CACHE

    f32 = np.float32
    wqT = _block_w_panels(np.asarray(w_q, dtype=f32).T, P)
    wkT = _block_w_panels(np.asarray(w_k, dtype=f32).T, P)
    wvT = _block_w_panels(np.asarray(w_v, dtype=f32).T, KC)
    bq = np.ascontiguousarray(np.asarray(b_q, dtype=f32))
    bk = np.ascontiguousarray(np.asarray(b_k, dtype=f32))
    bv = np.ascontiguousarray(np.asarray(b_v, dtype=f32))
    mask = np.asarray(mask, dtype=f32)
    # pre-scale so the kernel can add it to raw QK^T before the fused 1/sqrt(D)
    maskS_halves = [
        np.ascontiguousarray(mask[0, h * SQ:(h + 1) * SQ, :] * f32(math.sqrt(D)))
        for h in range(2)
    ]

    kT_b = [_block_xT(np.asarray(k[b], dtype=f32), KC) for b in range(B)]
    vT_b = [_block_xT(np.asarray(v[b], dtype=f32), P) for b in range(B)]

    in_maps = []
    for c in range(8):
        b, h = c // 2, c % 2
        rows = slice(h * SQ, (h + 1) * SQ)
        in_maps.append({
            "qT": _block_xT(np.asarray(q[b], dtype=f32)[rows, :], KC),
            "kT": kT_b[b],
            "vT": vT_b[b],
            "wqT": wqT, "wkT": wkT, "wvT": wvT,
            "bq": bq, "bk": bk, "bv": bv,
            "maskS": maskS_halves[h],
        })

    res = run_bass_kernel_spmd(nc, in_maps, list(range(8)))
    LAST_RESULT = res

    out = np.empty((B, S, D), dtype=f32)
    for c in range(8):
        b, h = c // 2, c % 2
        out[b, h * SQ:(h + 1) * SQ, :] = res.results[c]["out"]
    return out
